# revision 6
# baseline (speedup 1.0000x reference)
"""Trainium2 Bass kernel for the channel-attention module.

Reference computation (per batch item, C=256 channels, N=4096 pixels):
    q = wq@x + bq; k = wk@x + bk; v = wv@x + bv          (1x1 convs)
    energy = q @ k^T                 [C, C]
    attn = softmax(energy, -1)
    out = attn @ v                   [C, N]
    y = gamma*out + x

Algorithm used here (algebraically identical, far less PE work):
    G' = [[x x^T, s], [s^T, N]]  (s = row sums of x)  -- Gram matrix, 257x257
    energy = wq' G' wk'^T   where wq' = [wq | bq], wk' = [wk | bk]
    attn = softmax(energy)
    out = (attn wv) x + (attn bv) 1^T
    y = gamma*out + x

This eliminates materializing q, k, v entirely: the only O(C*C*N) work is
the Gram matrix, the transpose of x it needs, and the final (attn wv) x.

Sharding: data-parallel over batch B=16 across 8 cores (2 items/core).

Matmul dtype: float32r (TF32-like 10-bit-mantissa fp32 at full PE rate,
fp32 PSUM accumulation). Measured end-to-end error vs the fp32 reference
is ~2e-4 (fro) / ~1.5e-3 (elementwise max-rel).
"""

import os
import sys

sys.path.insert(0, "/opt/trn_rl_repo")

from contextlib import ExitStack

import numpy as np

import concourse.bacc as bacc
import concourse.tile as tile
from concourse import masks, mybir
from concourse.bass_utils import run_bass_kernel_spmd

F32 = mybir.dt.float32
F32R = mybir.dt.float32r

B, C, H, W = 16, 256, 64, 64
N = H * W                 # 4096
NCORES = 8
PB = B // NCORES          # batch items per core
P = 128                   # partitions
CT = C // P               # 2 channel tiles
NT = N // P               # 32 pixel tiles
FD = 512                  # free-dim chunk for the final matmul
NCH = N // FD             # 8 chunks
YCOLS = 2048              # output staging width (1 MiB per DMA)

# wpack column layout (all fp32, packed on host into [128, WCOLS]):
_WQ0, _WQ1 = 0, 256              # wq^T rows 0:128 / 128:256   [128,256] each
_WK0, _WK1 = 512, 768            # wk^T rows 0:128 / 128:256
_WV0, _WV1 = 1024, 1282          # [wv | bv | 0] rows 0:128 / 128:256 [128,258]
_BQ = 1540                       # row 0: bq                    [1,256]
_BK = 1796                       # row 0: bk                    [1,256]
_GA = 2052                       # gamma replicated             [128,1]
_ONE = 2053                      # col of 1.0 (all rows), col of 0.0
_NC = 2055                       # row 0: float(N) = 4096.0
WCOLS = 2056


def _emit_core_program(nc, tc, ctx, x_in, wpack, y_out):
    sb1 = ctx.enter_context(tc.tile_pool(name="sb1", bufs=1))
    xbp = ctx.enter_context(tc.tile_pool(name="xbp", bufs=2 * PB))
    xtp = ctx.enter_context(tc.tile_pool(name="xtp", bufs=6))
    gsb = ctx.enter_context(tc.tile_pool(name="gsb", bufs=4))
    smp = ctx.enter_context(tc.tile_pool(name="smp", bufs=8))
    ysp = ctx.enter_context(tc.tile_pool(name="ysp", bufs=4))
    # PSUM: 2+2+2+2 = 8 banks
    pst = ctx.enter_context(tc.tile_pool(name="pst", bufs=2, space="PSUM"))
    psg = ctx.enter_context(tc.tile_pool(name="psg", bufs=2, space="PSUM"))
    pss = ctx.enter_context(tc.tile_pool(name="pss", bufs=2, space="PSUM"))
    pso = ctx.enter_context(tc.tile_pool(name="pso", bufs=2, space="PSUM"))

    # --- constants: packed weights (one DMA) + identity ---
    wt = sb1.tile([P, WCOLS], F32R)
    nc.sync.dma_start(out=wt, in_=wpack[:, :].bitcast(F32R))
    ident_f = sb1.tile([P, P], F32)
    masks.make_identity(nc, ident_f[:, :])
    ident = sb1.tile([P, P], F32R)
    nc.vector.tensor_copy(ident, ident_f)

    gamma_col = wt[:, _GA:_GA + 1].bitcast(F32)

    wq_k = [wt[:, _WQ0:_WQ0 + 256], wt[:, _WQ1:_WQ1 + 256],
            wt[0:1, _BQ:_BQ + 256]]
    wk_k = [wt[:, _WK0:_WK0 + 256], wt[:, _WK1:_WK1 + 256],
            wt[0:1, _BK:_BK + 256]]
    wv_t = [wt[:, _WV0:_WV0 + 258], wt[:, _WV1:_WV1 + 258]]

    for b in range(PB):
        # ---- load x (2 tiles of [128, 4096]) ----
        xb = []
        for ct in range(CT):
            t = xbp.tile([P, N], F32R, tag="xb")
            nc.sync.dma_start(
                out=t, in_=x_in[b, ct * P:(ct + 1) * P, :].bitcast(F32R))
            xb.append(t)

        # ---- transpose x and accumulate Gram matrix ----
        # xt tiles are [128 (n), 257]: cols 0:256 = x^T block, col 256 = 1.0
        # G psum [128 (c), 257]: cols 0:256 = G block, col 256 = row sums s
        with nc.named_scope("gram"):
            gps = [psg.tile([P, 258], F32, tag="gacc", name=f"gps{i}")
                   for i in range(CT)]
            for nt in range(NT):
                xt = xtp.tile([P, 258], F32R, tag="xt")
                for ct in range(CT):
                    tp = pst.tile([P, P], F32R, tag="tp")
                    nc.tensor.transpose(
                        tp, xb[ct][:, nt * P:(nt + 1) * P], ident)
                    if (nt + ct) % 2 == 0:
                        nc.vector.tensor_copy(
                            xt[:, ct * P:(ct + 1) * P], tp)
                    else:
                        nc.scalar.copy(xt[:, ct * P:(ct + 1) * P], tp)
                nc.gpsimd.tensor_copy(xt[:, 256:258], wt[:, _ONE:_ONE + 2])
                for ct in range(CT):
                    nc.tensor.matmul(
                        gps[ct], xt[:, ct * P:(ct + 1) * P], xt[:, 0:258],
                        start=(nt == 0), stop=(nt == NT - 1))

            # G' sbuf tiles g0,g1 [128, 257] (rows 0:256) + g2 [1, 257]
            g = []
            for ct in range(CT):
                gt = gsb.tile([P, 257], F32R, tag="g")
                nc.vector.tensor_copy(gt, gps[ct][:, 0:257])
                g.append(gt)
            g2 = gsb.tile([1, 257], F32R, tag="g2")
            for ct in range(CT):
                sp = pst.tile([1, P], F32R, tag="tp")
                nc.tensor.transpose(sp, g[ct][:, 256:257], ident)
                nc.vector.tensor_copy(g2[0:1, ct * P:(ct + 1) * P], sp)
            nc.vector.tensor_copy(g2[0:1, 256:257], wt[0:1, _NC:_NC + 1])
        gk = [g[0], g[1], g2]

        # ---- T^T = G' wq'^T   [257, 256] ----
        with nc.named_scope("energy"):
            ttp = [pss.tile([P, 256], F32, tag="big", name=f"ttp{i}")
                   for i in range(CT)]
            ttp.append(pss.tile([1, 256], F32, tag="big", name="ttp2"))
            for mt in range(CT):          # output rows m 0:128 / 128:256
                for kt in range(3):       # contraction over p
                    lhs = gk[kt][:, mt * P:(mt + 1) * P]
                    nc.tensor.matmul(ttp[mt], lhs, wq_k[kt],
                                     start=(kt == 0), stop=(kt == 2))
            for kt in range(3):           # output row 256 (lhsT = s column)
                lhs = gk[kt][:, 256:257]
                nc.tensor.matmul(ttp[2], lhs, wq_k[kt],
                                 start=(kt == 0), stop=(kt == 2))
            tt = []
            for mt in range(CT):
                t = gsb.tile([P, 256], F32R, tag="tt")
                nc.vector.tensor_copy(t, ttp[mt])
                tt.append(t)
            t2 = gsb.tile([1, 256], F32R, tag="tt2")
            nc.vector.tensor_copy(t2, ttp[2])
            tt.append(t2)

            # ---- E = T wk'^T : e [128, 512] = [E(i0) | E(i1)] ----
            ep = pss.tile([P, 2 * 256], F32, tag="big")
            for it in range(CT):
                for kt in range(3):
                    lhs = tt[kt][:, it * P:(it + 1) * P] if kt < 2 \
                        else tt[2][0:1, it * P:(it + 1) * P]
                    nc.tensor.matmul(ep[:, it * 256:(it + 1) * 256],
                                     lhs, wk_k[kt],
                                     start=(kt == 0), stop=(kt == 2))

        # ---- softmax over free dim of each half ----
        with nc.named_scope("softmax"):
            attn = []
            for it in range(CT):
                eslice = ep[:, it * 256:(it + 1) * 256]
                nmx = smp.tile([P, 1], F32, tag="nmx")
                nc.vector.tensor_reduce(
                    nmx, eslice, axis=mybir.AxisListType.X,
                    op=mybir.AluOpType.max, negate=True)
                at = smp.tile([P, 256], F32R, tag="attn")
                rs = smp.tile([P, 1], F32, tag="rs")
                nc.scalar.activation(
                    out=at, in_=eslice,
                    func=mybir.ActivationFunctionType.Exp,
                    bias=nmx, scale=1.0, accum_out=rs)
                ri = smp.tile([P, 1], F32, tag="ri")
                nc.vector.reciprocal(ri, rs)
                nc.vector.tensor_scalar_mul(at, at, ri)
                attn.append(at)

            # ---- attn^T  (4 PE transposes) ----
            attnT = []
            for jt in range(CT):
                aT = smp.tile([P, 256], F32R, tag="attnT")
                for it in range(CT):
                    tp = pst.tile([P, P], F32R, tag="tp")
                    nc.tensor.transpose(
                        tp, attn[it][:, jt * P:(jt + 1) * P], ident)
                    nc.vector.tensor_copy(aT[:, it * P:(it + 1) * P], tp)
                attnT.append(aT)

        # ---- A'^T = wv' attn^T, pre-scaled by gamma ----
        # a [128, 512] = [A^T(c0) | A^T(c1)], arow [1,256] = (attn bv)^T
        with nc.named_scope("attn_wv"):
            ap_ = pss.tile([P, 2 * 256], F32, tag="big")
            for mt in range(CT):
                for jt in range(CT):
                    nc.tensor.matmul(
                        ap_[:, mt * 256:(mt + 1) * 256],
                        wv_t[jt][:, mt * P:(mt + 1) * P], attnT[jt],
                        start=(jt == 0), stop=(jt == 1))
            arow = pss.tile([2, 256], F32, tag="big")
            for jt in range(CT):
                nc.tensor.matmul(arow, wv_t[jt][:, 256:258], attnT[jt],
                                 start=(jt == 0), stop=(jt == 1))
            at_s = []
            for mt in range(CT):
                t = gsb.tile([P, 256], F32R, tag="ats")
                nc.vector.tensor_scalar_mul(
                    t, ap_[:, mt * 256:(mt + 1) * 256], gamma_col)
                at_s.append(t)
            abv_r = smp.tile([2, 256], F32R, tag="abvr")
            nc.vector.tensor_scalar_mul(abv_r, arow, gamma_col[0:2, :])
            gabv = []
            for it in range(CT):
                tp = pst.tile([P, 2], F32R, tag="tp")
                nc.tensor.transpose(
                    tp, abv_r[0:2, it * P:(it + 1) * P], ident[0:2, 0:2])
                gc = smp.tile([P, 1], F32, tag="gabv")
                nc.vector.tensor_copy(gc, tp[:, 0:1])
                gabv.append(gc)

        # ---- out = gamma*(A x) ; y = out + gamma*abv + x ----
        with nc.named_scope("out_mm"):
            for it in range(CT):
                for yg in range(N // YCOLS):
                    ysb = ysp.tile([P, YCOLS], F32, tag="ysb")
                    for sub in range(YCOLS // FD):
                        nch = yg * (YCOLS // FD) + sub
                        op = pso.tile([P, FD], F32, tag="out")
                        for ct in range(CT):
                            nc.tensor.matmul(
                                op, at_s[ct][:, it * P:(it + 1) * P],
                                xb[ct][:, nch * FD:(nch + 1) * FD],
                                start=(ct == 0), stop=(ct == CT - 1))
                        # y = (psum + gamma*abv) + x   (one DVE op)
                        nc.vector.scalar_tensor_tensor(
                            out=ysb[:, sub * FD:(sub + 1) * FD],
                            in0=op, scalar=gabv[it],
                            in1=xb[it][:, nch * FD:(nch + 1) * FD]
                            .bitcast(F32),
                            op0=mybir.AluOpType.add,
                            op1=mybir.AluOpType.add)
                    nc.sync.dma_start(
                        out=y_out[b, it * P:(it + 1) * P,
                                  yg * YCOLS:(yg + 1) * YCOLS],
                        in_=ysb)


_CACHE = {}
LAST_RESULTS = None


def _build():
    if "nc" in _CACHE:
        return _CACHE["nc"]
    nc = bacc.Bacc()
    x_in = nc.declare_dram_parameter("x", [PB, C, N], F32, isOutput=False)
    wpack = nc.declare_dram_parameter("wpack", [P, WCOLS], F32,
                                      isOutput=False)
    y_out = nc.declare_dram_parameter("y", [PB, C, N], F32, isOutput=True)
    with ExitStack() as ctx:
        tc = ctx.enter_context(tile.TileContext(nc))
        _emit_core_program(nc, tc, ctx, x_in, wpack, y_out)
    nc.compile()
    _CACHE["nc"] = nc
    return nc


def _pack_weights(wq, bq, wk, bk, wv, bv, gamma):
    wp = np.zeros((P, WCOLS), np.float32)
    wqT = np.ascontiguousarray(wq.T)
    wkT = np.ascontiguousarray(wk.T)
    wp[:, _WQ0:_WQ0 + 256] = wqT[0:P]
    wp[:, _WQ1:_WQ1 + 256] = wqT[P:C]
    wp[:, _WK0:_WK0 + 256] = wkT[0:P]
    wp[:, _WK1:_WK1 + 256] = wkT[P:C]
    wvp = np.concatenate([wv, bv[:, None],
                          np.zeros((C, 1), np.float32)], axis=1)  # [256, 258]
    wp[:, _WV0:_WV0 + 258] = wvp[0:P]
    wp[:, _WV1:_WV1 + 258] = wvp[P:C]
    wp[0, _BQ:_BQ + 256] = bq
    wp[0, _BK:_BK + 256] = bk
    wp[:, _GA] = np.float32(gamma)
    wp[:, _ONE] = 1.0
    wp[0, _NC] = float(N)
    return wp


def kernel(x, wq, bq, wk, bk, wv, bv, gamma):
    global LAST_RESULTS
    x = np.ascontiguousarray(np.asarray(x, np.float32))
    xf = x.reshape(B, C, N)
    wp = _pack_weights(np.asarray(wq, np.float32), np.asarray(bq, np.float32),
                       np.asarray(wk, np.float32), np.asarray(bk, np.float32),
                       np.asarray(wv, np.float32), np.asarray(bv, np.float32),
                       np.asarray(gamma, np.float32).reshape(-1)[0])
    nc = _build()
    in_maps = []
    for k in range(NCORES):
        in_maps.append({
            "x": np.ascontiguousarray(xf[k * PB:(k + 1) * PB]),
            "wpack": wp,
        })
    trace = bool(int(os.environ.get("KERNEL_TRACE", "0")))
    res = run_bass_kernel_spmd(nc, in_maps, core_ids=list(range(NCORES)),
                               trace=trace)
    LAST_RESULTS = res
    y = np.concatenate([res.results[k]["y"][None] for k in range(NCORES)],
                       axis=0)
    return y.reshape(B, C, H, W).astype(np.float32)


# revision 10
# speedup vs baseline: 1.1886x; 1.1886x over previous
"""Trainium2 Bass kernel for the channel-attention module.

Reference computation (per batch item, C=256 channels, N=4096 pixels):
    q = wq@x + bq; k = wk@x + bk; v = wv@x + bv          (1x1 convs)
    energy = q @ k^T                 [C, C]
    attn = softmax(energy, -1)
    out = attn @ v                   [C, N]
    y = gamma*out + x

Algorithm used here (algebraically identical, far less PE work):
    G' = [[x x^T, s], [s^T, N]]  (s = row sums of x)  -- Gram matrix, 257x257
    energy = wq' G' wk'^T   where wq' = [wq | bq], wk' = [wk | bk]
    attn = softmax(energy)
    out = (attn wv) x + (attn bv) 1^T
    y = gamma*out + x

This eliminates materializing q, k, v entirely: the only O(C*C*N) work is
the Gram matrix, the transpose of x it needs, and the final (attn wv) x.

Sharding: data-parallel over batch B=16 across 8 cores (2 items/core).

Matmul dtype: float16 (10-bit mantissa like TF32, fp32 PSUM accumulation,
full PE rate + fast weight load). Measured end-to-end error vs the fp32
reference: ~3e-4 (fro). All intermediates stay well inside fp16 range
(|G|<4.4e3, |energy|<450).
"""

import os
import sys

sys.path.insert(0, "/opt/trn_rl_repo")

from contextlib import ExitStack

import numpy as np

import concourse.bacc as bacc
import concourse.tile as tile
from concourse import masks, mybir
from concourse.bass_utils import run_bass_kernel_spmd

F32 = mybir.dt.float32
F16 = mybir.dt.float16

B, C, H, W = 16, 256, 64, 64
N = H * W                 # 4096
NCORES = 8
PB = B // NCORES          # batch items per core
P = 128                   # partitions
CT = C // P               # 2 channel tiles
NT = N // P               # 32 pixel tiles
FD = 512                  # free-dim chunk for the final matmul
YCOLS = 2048              # output staging width (1 MiB per DMA)

# wpack column layout (fp16, packed on host into [128, WCOLS]):
_WQ0, _WQ1 = 0, 256              # wq^T rows 0:128 / 128:256   [128,256] each
_WK0, _WK1 = 512, 768            # wk^T rows 0:128 / 128:256
_WV0, _WV1 = 1024, 1282         # [wv | bv | 0] rows 0:128 / 128:256 [128,258]
_BQ = 1540                       # rows 0:2: [bq; 0]            [2,256]
_BK = 1796                       # rows 0:2: [bk; 0]            [2,256]
_GA = 2052                       # gamma replicated             [128,1]
_ONE = 2053                      # col of 1.0 (all rows), col of 0.0
_NC = 2055                       # row 0: float(N) = 4096.0
WCOLS = 2056


def _emit_core_program(nc, tc, ctx, x_in, wpack, y_out):
    sb1 = ctx.enter_context(tc.tile_pool(name="sb1", bufs=1))
    xbp = ctx.enter_context(tc.tile_pool(name="xbp", bufs=2 * PB))
    xtp = ctx.enter_context(tc.tile_pool(name="xtp", bufs=6))
    gsb = ctx.enter_context(tc.tile_pool(name="gsb", bufs=4))
    smp = ctx.enter_context(tc.tile_pool(name="smp", bufs=8))
    ysp = ctx.enter_context(tc.tile_pool(name="ysp", bufs=4))
    # PSUM: 2+2+2+2 = 8 banks
    pst = ctx.enter_context(tc.tile_pool(name="pst", bufs=2, space="PSUM"))
    psg = ctx.enter_context(tc.tile_pool(name="psg", bufs=2, space="PSUM"))
    pss = ctx.enter_context(tc.tile_pool(name="pss", bufs=2, space="PSUM"))
    pso = ctx.enter_context(tc.tile_pool(name="pso", bufs=2, space="PSUM"))

    # --- constants: packed weights (one DMA) + identity ---
    wt = sb1.tile([P, WCOLS], F16)
    nc.sync.dma_start(out=wt, in_=wpack[:, :])
    ident_f = sb1.tile([P, P], F32)
    masks.make_identity(nc, ident_f[:, :])
    ident = sb1.tile([P, P], F16)
    nc.vector.tensor_copy(ident, ident_f)

    gamma_col = sb1.tile([P, 1], F32, name="gamma_col")
    nc.vector.tensor_copy(gamma_col, wt[:, _GA:_GA + 1])

    wq_k = [wt[:, _WQ0:_WQ0 + 256], wt[:, _WQ1:_WQ1 + 256],
            wt[0:2, _BQ:_BQ + 256]]
    wk_k = [wt[:, _WK0:_WK0 + 256], wt[:, _WK1:_WK1 + 256],
            wt[0:2, _BK:_BK + 256]]
    wv_t = [wt[:, _WV0:_WV0 + 258], wt[:, _WV1:_WV1 + 258]]

    for b in range(PB):
        # ---- load x (2 tiles of [128, 4096] fp16) ----
        xb = []
        for ct in range(CT):
            t = xbp.tile([P, N], F16, tag="xb")
            nc.sync.dma_start(out=t, in_=x_in[b, ct * P:(ct + 1) * P, :])
            xb.append(t)

        # ---- transpose x and accumulate Gram matrix ----
        # xt tiles are [128 (n), 258]: cols 0:256 = x^T, col 256 = 1, 257 = 0
        # G psum [128 (c), 258]: cols 0:256 = G block, col 256 = row sums s
        with nc.named_scope("gram"):
            gps = [psg.tile([P, 258], F32, tag="gacc", name=f"gps{i}")
                   for i in range(CT)]
            for nt in range(NT):
                xt = xtp.tile([P, 258], F16, tag="xt")
                for ct in range(CT):
                    tp = pst.tile([P, P], F16, tag="tp")
                    nc.tensor.transpose(
                        tp, xb[ct][:, nt * P:(nt + 1) * P], ident)
                    if (nt + ct) % 2 == 0:
                        nc.vector.tensor_copy(
                            xt[:, ct * P:(ct + 1) * P], tp)
                    else:
                        nc.scalar.copy(xt[:, ct * P:(ct + 1) * P], tp)
                nc.gpsimd.tensor_copy(xt[:, 256:258], wt[:, _ONE:_ONE + 2])
                for ct in range(CT):
                    nc.tensor.matmul(
                        gps[ct], xt[:, ct * P:(ct + 1) * P], xt[:, 0:258],
                        start=(nt == 0), stop=(nt == NT - 1))

            # G' sbuf tiles g0,g1 [128, 257] (rows 0:256) + g2 [2, 257]
            g = []
            for ct in range(CT):
                gt = gsb.tile([P, 258], F16, tag="g")
                nc.vector.tensor_copy(gt, gps[ct])
                g.append(gt)
            g2 = gsb.tile([2, 257], F16, tag="g2")
            for ct in range(CT):
                sp = pst.tile([2, P], F16, tag="tp")
                nc.tensor.transpose(sp, g[ct][:, 256:258], ident)
                nc.vector.tensor_copy(g2[0:2, ct * P:(ct + 1) * P], sp)
            nc.vector.tensor_copy(g2[0:1, 256:257], wt[0:1, _NC:_NC + 1])
        gk = [g[0], g[1], g2]

        # ---- T^T = G' wq'^T   [257, 256] ----
        with nc.named_scope("energy"):
            ttp = [pss.tile([P, 256], F32, tag="big", name=f"ttp{i}")
                   for i in range(CT)]
            ttp.append(pss.tile([2, 256], F32, tag="big", name="ttp2"))
            for mt in range(CT):          # output rows m 0:128 / 128:256
                for kt in range(3):       # contraction over p
                    lhs = gk[kt][:, mt * P:(mt + 1) * P]
                    nc.tensor.matmul(ttp[mt], lhs, wq_k[kt],
                                     start=(kt == 0), stop=(kt == 2))
            for kt in range(3):           # output rows 256:257 (lhsT = s col)
                lhs = gk[kt][:, 256:257]
                nc.tensor.matmul(ttp[2][0:1, :], lhs, wq_k[kt],
                                 start=(kt == 0), stop=(kt == 2))
            tt = []
            for mt in range(CT):
                t = gsb.tile([P, 256], F16, tag="tt")
                nc.vector.tensor_copy(t, ttp[mt])
                tt.append(t)
            t2 = gsb.tile([1, 256], F16, tag="tt2")
            nc.vector.tensor_copy(t2, ttp[2][0:1, :])
            tt.append(t2)

            # ---- E = T wk'^T : e [128, 512] = [E(i0) | E(i1)] ----
            ep = pss.tile([P, 2 * 256], F32, tag="big")
            for it in range(CT):
                for kt in range(3):
                    lhs = tt[kt][:, it * P:(it + 1) * P] if kt < 2 \
                        else tt[2][0:1, it * P:(it + 1) * P]
                    nc.tensor.matmul(ep[:, it * 256:(it + 1) * 256],
                                     lhs, wk_k[kt][0:1, :] if kt == 2
                                     else wk_k[kt],
                                     start=(kt == 0), stop=(kt == 2))

        # ---- softmax over free dim of each half ----
        with nc.named_scope("softmax"):
            attn = []
            for it in range(CT):
                eslice = ep[:, it * 256:(it + 1) * 256]
                nmx = smp.tile([P, 1], F32, tag="nmx")
                nc.vector.tensor_reduce(
                    nmx, eslice, axis=mybir.AxisListType.X,
                    op=mybir.AluOpType.max, negate=True)
                at = smp.tile([P, 256], F16, tag="attn")
                rs = smp.tile([P, 1], F32, tag="rs")
                nc.scalar.activation(
                    out=at, in_=eslice,
                    func=mybir.ActivationFunctionType.Exp,
                    bias=nmx, scale=1.0, accum_out=rs)
                ri = smp.tile([P, 1], F32, tag="ri")
                nc.vector.reciprocal(ri, rs)
                nc.vector.tensor_scalar_mul(at, at, ri)
                attn.append(at)

            # ---- attn^T  (4 PE transposes) ----
            attnT = []
            for jt in range(CT):
                aT = smp.tile([P, 256], F16, tag="attnT")
                for it in range(CT):
                    tp = pst.tile([P, P], F16, tag="tp")
                    nc.tensor.transpose(
                        tp, attn[it][:, jt * P:(jt + 1) * P], ident)
                    nc.vector.tensor_copy(aT[:, it * P:(it + 1) * P], tp)
                attnT.append(aT)

        # ---- A'^T = wv' attn^T, pre-scaled by gamma ----
        # a [128, 512] = [A^T(c0) | A^T(c1)], arow [2,256] = [(attn bv)^T; 0]
        with nc.named_scope("attn_wv"):
            ap_ = pss.tile([P, 2 * 256], F32, tag="big")
            for mt in range(CT):
                for jt in range(CT):
                    nc.tensor.matmul(
                        ap_[:, mt * 256:(mt + 1) * 256],
                        wv_t[jt][:, mt * P:(mt + 1) * P], attnT[jt],
                        start=(jt == 0), stop=(jt == 1))
            arow = pss.tile([2, 256], F32, tag="big")
            for jt in range(CT):
                nc.tensor.matmul(arow, wv_t[jt][:, 256:258], attnT[jt],
                                 start=(jt == 0), stop=(jt == 1))
            at_s = []
            for mt in range(CT):
                t = gsb.tile([P, 256], F16, tag="ats")
                nc.vector.tensor_scalar_mul(
                    t, ap_[:, mt * 256:(mt + 1) * 256], gamma_col)
                at_s.append(t)
            abv_r = smp.tile([2, 256], F16, tag="abvr")
            nc.vector.tensor_scalar_mul(abv_r, arow, gamma_col[0:2, :])
            gabv = []
            for it in range(CT):
                tp = pst.tile([P, 2], F16, tag="tp")
                nc.tensor.transpose(
                    tp, abv_r[0:2, it * P:(it + 1) * P], ident[0:2, 0:2])
                gc = smp.tile([P, 1], F32, tag="gabv")
                nc.vector.tensor_copy(gc, tp[:, 0:1])
                gabv.append(gc)

        # ---- out = gamma*(A x) ; y = out + gamma*abv + x ----
        with nc.named_scope("out_mm"):
            for it in range(CT):
                for yg in range(N // YCOLS):
                    ysb = ysp.tile([P, YCOLS], F32, tag="ysb")
                    for sub in range(YCOLS // FD):
                        nch = yg * (YCOLS // FD) + sub
                        op = pso.tile([P, FD], F32, tag="out")
                        for ct in range(CT):
                            nc.tensor.matmul(
                                op, at_s[ct][:, it * P:(it + 1) * P],
                                xb[ct][:, nch * FD:(nch + 1) * FD],
                                start=(ct == 0), stop=(ct == CT - 1))
                        # y = (psum + gamma*abv) + x   (one DVE op)
                        nc.vector.scalar_tensor_tensor(
                            out=ysb[:, sub * FD:(sub + 1) * FD],
                            in0=op, scalar=gabv[it],
                            in1=xb[it][:, nch * FD:(nch + 1) * FD],
                            op0=mybir.AluOpType.add,
                            op1=mybir.AluOpType.add)
                    nc.sync.dma_start(
                        out=y_out[b, it * P:(it + 1) * P,
                                  yg * YCOLS:(yg + 1) * YCOLS],
                        in_=ysb)


_CACHE = {}
LAST_RESULTS = None


def _build():
    if "nc" in _CACHE:
        return _CACHE["nc"]
    nc = bacc.Bacc()
    x_in = nc.declare_dram_parameter("x", [PB, C, N], F16, isOutput=False)
    wpack = nc.declare_dram_parameter("wpack", [P, WCOLS], F16,
                                      isOutput=False)
    y_out = nc.declare_dram_parameter("y", [PB, C, N], F32, isOutput=True)
    with ExitStack() as ctx:
        tc = ctx.enter_context(tile.TileContext(nc))
        _emit_core_program(nc, tc, ctx, x_in, wpack, y_out)
    nc.compile()
    _CACHE["nc"] = nc
    return nc


def _pack_weights(wq, bq, wk, bk, wv, bv, gamma):
    wp = np.zeros((P, WCOLS), np.float16)
    wqT = np.ascontiguousarray(wq.T).astype(np.float16)
    wkT = np.ascontiguousarray(wk.T).astype(np.float16)
    wp[:, _WQ0:_WQ0 + 256] = wqT[0:P]
    wp[:, _WQ1:_WQ1 + 256] = wqT[P:C]
    wp[:, _WK0:_WK0 + 256] = wkT[0:P]
    wp[:, _WK1:_WK1 + 256] = wkT[P:C]
    wvp = np.concatenate([wv, bv[:, None]],
                         axis=1).astype(np.float16)  # [256, 257]
    wp[:, _WV0:_WV0 + 257] = wvp[0:P]
    wp[:, _WV1:_WV1 + 257] = wvp[P:C]
    wp[0, _BQ:_BQ + 256] = bq.astype(np.float16)
    wp[0, _BK:_BK + 256] = bk.astype(np.float16)
    wp[:, _GA] = np.float16(gamma)
    wp[:, _ONE] = np.float16(1.0)
    wp[0, _NC] = np.float16(float(N))
    return wp


def kernel(x, wq, bq, wk, bk, wv, bv, gamma):
    global LAST_RESULTS
    x = np.asarray(x, np.float32)
    x16 = np.ascontiguousarray(x.reshape(B, C, N).astype(np.float16))
    wp = _pack_weights(np.asarray(wq, np.float32), np.asarray(bq, np.float32),
                       np.asarray(wk, np.float32), np.asarray(bk, np.float32),
                       np.asarray(wv, np.float32), np.asarray(bv, np.float32),
                       np.asarray(gamma, np.float32).reshape(-1)[0])
    nc = _build()
    in_maps = []
    for k in range(NCORES):
        in_maps.append({
            "x": np.ascontiguousarray(x16[k * PB:(k + 1) * PB]),
            "wpack": wp,
        })
    trace = bool(int(os.environ.get("KERNEL_TRACE", "0")))
    res = run_bass_kernel_spmd(nc, in_maps, core_ids=list(range(NCORES)),
                               trace=trace)
    LAST_RESULTS = res
    y = np.concatenate([res.results[k]["y"][None] for k in range(NCORES)],
                       axis=0)
    return y.reshape(B, C, H, W).astype(np.float32)


# revision 11
# speedup vs baseline: 1.7997x; 1.5140x over previous
"""Trainium2 Bass kernel for the channel-attention module.

Reference computation (per batch item, C=256 channels, N=4096 pixels):
    q = wq@x + bq; k = wk@x + bk; v = wv@x + bv          (1x1 convs)
    energy = q @ k^T                 [C, C]
    attn = softmax(energy, -1)
    out = attn @ v                   [C, N]
    y = gamma*out + x

Algorithm used here (algebraically identical, far less PE work):
    G' = [[x x^T, s], [s^T, N]]  (s = row sums of x)  -- Gram matrix, 257x257
    energy = wq' G' wk'^T   where wq' = [wq | bq], wk' = [wk | bk]
    attn = softmax(energy)
    out = (attn wv) x + (attn bv) 1^T
    y = gamma*out + x

This eliminates materializing q, k, v entirely: the only O(C*C*N) work is
the Gram matrix, the transpose of x it needs, and the final (attn wv) x.

Sharding: data-parallel over batch B=16 across 8 cores (2 items/core).

Matmul dtype: float16 (10-bit mantissa like TF32, fp32 PSUM accumulation,
full PE rate + fast weight load). Measured end-to-end error vs the fp32
reference: ~3e-4 (fro). All intermediates stay well inside fp16 range
(|G|<4.4e3, |energy|<450).
"""

import os
import sys

sys.path.insert(0, "/opt/trn_rl_repo")

from contextlib import ExitStack

import numpy as np

import concourse.bacc as bacc
import concourse.tile as tile
from concourse import masks, mybir
from concourse.bass_utils import run_bass_kernel_spmd

F32 = mybir.dt.float32
F16 = mybir.dt.float16

B, C, H, W = 16, 256, 64, 64
N = H * W                 # 4096
NCORES = 8
PB = B // NCORES          # batch items per core
P = 128                   # partitions
CT = C // P               # 2 channel tiles
NT = N // P               # 32 pixel tiles
FD = 512                  # free-dim chunk for the final matmul
YCOLS = 2048              # output staging width (1 MiB per DMA)

# wpack column layout (fp16, packed on host into [128, WCOLS]):
_WQ0, _WQ1 = 0, 256              # wq^T rows 0:128 / 128:256   [128,256] each
_WK0, _WK1 = 512, 768            # wk^T rows 0:128 / 128:256
_WV0, _WV1 = 1024, 1282         # [wv | bv | 0] rows 0:128 / 128:256 [128,258]
_BQ = 1540                       # rows 0:2: [bq; 0]            [2,256]
_BK = 1796                       # rows 0:2: [bk; 0]            [2,256]
_GA = 2052                       # gamma replicated             [128,1]
_ONE = 2053                      # col of 1.0 (all rows), col of 0.0
_NC = 2055                       # row 0: float(N) = 4096.0
WCOLS = 2056


def _emit_core_program(nc, tc, ctx, x_in, wpack, y_out):
    sb1 = ctx.enter_context(tc.tile_pool(name="sb1", bufs=1))
    xbp = ctx.enter_context(tc.tile_pool(name="xbp", bufs=2 * PB))
    xtp = ctx.enter_context(tc.tile_pool(name="xtp", bufs=2))
    gsb = ctx.enter_context(tc.tile_pool(name="gsb", bufs=4))
    smp = ctx.enter_context(tc.tile_pool(name="smp", bufs=8))
    ysp = ctx.enter_context(tc.tile_pool(name="ysp", bufs=4))
    # PSUM: 2+2+2+2 = 8 banks
    pst = ctx.enter_context(tc.tile_pool(name="pst", bufs=2, space="PSUM"))
    psg = ctx.enter_context(tc.tile_pool(name="psg", bufs=2, space="PSUM"))
    pss = ctx.enter_context(tc.tile_pool(name="pss", bufs=2, space="PSUM"))
    pso = ctx.enter_context(tc.tile_pool(name="pso", bufs=2, space="PSUM"))

    # --- constants: packed weights (one DMA) + identity ---
    wt = sb1.tile([P, WCOLS], F16)
    nc.sync.dma_start(out=wt, in_=wpack[:, :])
    ident_f = sb1.tile([P, P], F32)
    masks.make_identity(nc, ident_f[:, :])
    ident = sb1.tile([P, P], F16)
    nc.vector.tensor_copy(ident, ident_f)

    gamma_col = sb1.tile([P, 1], F32, name="gamma_col")
    nc.vector.tensor_copy(gamma_col, wt[:, _GA:_GA + 1])

    wq_k = [wt[:, _WQ0:_WQ0 + 256], wt[:, _WQ1:_WQ1 + 256],
            wt[0:2, _BQ:_BQ + 256]]
    wk_k = [wt[:, _WK0:_WK0 + 256], wt[:, _WK1:_WK1 + 256],
            wt[0:2, _BK:_BK + 256]]
    wv_t = [wt[:, _WV0:_WV0 + 258], wt[:, _WV1:_WV1 + 258]]

    for b in range(PB):
        # ---- load x (2 tiles of [128, 4096] fp16) ----
        xb = []
        for ct in range(CT):
            t = xbp.tile([P, N], F16, tag="xb")
            nc.sync.dma_start(out=t, in_=x_in[b, ct * P:(ct + 1) * P, :])
            xb.append(t)

        # ---- x^T via DMA-crossbar transpose, then the Gram matrix ----
        # xt_big [128, 32*256]: block t holds x^T[t*128:(t+1)*128, :] (fp16)
        # G psum [128 (c), 256] per ct; s = row sums via DVE reduce
        with nc.named_scope("gram"):
            xt = xtp.tile([P, NT * 256], F16, tag="xt")
            xt3 = xt.rearrange("p (t c) -> p t c", c=256)
            for gch in range(4):
                nc.sync.dma_start_transpose(
                    xt3[:, gch * (NT // 4):(gch + 1) * (NT // 4), :],
                    x_in[b, :, gch * (N // 4):(gch + 1) * (N // 4)])
            scol = []
            for ct in range(CT):
                sc = smp.tile([P, 1], F32, tag="scol")
                nc.vector.tensor_reduce(
                    sc, xb[ct], axis=mybir.AxisListType.X,
                    op=mybir.AluOpType.add)
                scol.append(sc)
            gps = [psg.tile([P, 256], F32, tag="gacc", name=f"gps{i}")
                   for i in range(CT)]
            for nt in range(NT):
                for ct in range(CT):
                    nc.tensor.matmul(
                        gps[ct],
                        xt[:, nt * 256 + ct * P:nt * 256 + (ct + 1) * P],
                        xt[:, nt * 256:(nt + 1) * 256],
                        start=(nt == 0), stop=(nt == NT - 1))

            # G' sbuf tiles g0,g1 [128, 258] (cols: G | s | 0) + g2 [2, 257]
            g = []
            for ct in range(CT):
                gt = gsb.tile([P, 258], F16, tag="g")
                nc.vector.tensor_copy(gt[:, 0:256], gps[ct])
                nc.vector.tensor_copy(gt[:, 256:257], scol[ct])
                nc.gpsimd.tensor_copy(gt[:, 257:258],
                                      wt[:, _ONE + 1:_ONE + 2])
                g.append(gt)
            g2 = gsb.tile([2, 257], F16, tag="g2")
            for ct in range(CT):
                sp = pst.tile([2, P], F16, tag="tp")
                nc.tensor.transpose(sp, g[ct][:, 256:258], ident)
                nc.vector.tensor_copy(g2[0:2, ct * P:(ct + 1) * P], sp)
            nc.vector.tensor_copy(g2[0:1, 256:257], wt[0:1, _NC:_NC + 1])
        gk = [g[0], g[1], g2]

        # ---- T^T = G' wq'^T   [257, 256] ----
        with nc.named_scope("energy"):
            ttp = [pss.tile([P, 256], F32, tag="big", name=f"ttp{i}")
                   for i in range(CT)]
            ttp.append(pss.tile([2, 256], F32, tag="big", name="ttp2"))
            for mt in range(CT):          # output rows m 0:128 / 128:256
                for kt in range(3):       # contraction over p
                    lhs = gk[kt][:, mt * P:(mt + 1) * P]
                    nc.tensor.matmul(ttp[mt], lhs, wq_k[kt],
                                     start=(kt == 0), stop=(kt == 2))
            for kt in range(3):           # output rows 256:257 (lhsT = s col)
                lhs = gk[kt][:, 256:257]
                nc.tensor.matmul(ttp[2][0:1, :], lhs, wq_k[kt],
                                 start=(kt == 0), stop=(kt == 2))
            tt = []
            for mt in range(CT):
                t = gsb.tile([P, 256], F16, tag="tt")
                nc.vector.tensor_copy(t, ttp[mt])
                tt.append(t)
            t2 = gsb.tile([1, 256], F16, tag="tt2")
            nc.vector.tensor_copy(t2, ttp[2][0:1, :])
            tt.append(t2)

            # ---- E = T wk'^T : e [128, 512] = [E(i0) | E(i1)] ----
            ep = pss.tile([P, 2 * 256], F32, tag="big")
            for it in range(CT):
                for kt in range(3):
                    lhs = tt[kt][:, it * P:(it + 1) * P] if kt < 2 \
                        else tt[2][0:1, it * P:(it + 1) * P]
                    nc.tensor.matmul(ep[:, it * 256:(it + 1) * 256],
                                     lhs, wk_k[kt][0:1, :] if kt == 2
                                     else wk_k[kt],
                                     start=(kt == 0), stop=(kt == 2))

        # ---- softmax over free dim of each half ----
        with nc.named_scope("softmax"):
            attn = []
            for it in range(CT):
                eslice = ep[:, it * 256:(it + 1) * 256]
                nmx = smp.tile([P, 1], F32, tag="nmx")
                nc.vector.tensor_reduce(
                    nmx, eslice, axis=mybir.AxisListType.X,
                    op=mybir.AluOpType.max, negate=True)
                at = smp.tile([P, 256], F16, tag="attn")
                rs = smp.tile([P, 1], F32, tag="rs")
                nc.scalar.activation(
                    out=at, in_=eslice,
                    func=mybir.ActivationFunctionType.Exp,
                    bias=nmx, scale=1.0, accum_out=rs)
                ri = smp.tile([P, 1], F32, tag="ri")
                nc.vector.reciprocal(ri, rs)
                nc.vector.tensor_scalar_mul(at, at, ri)
                attn.append(at)

            # ---- attn^T  (4 PE transposes) ----
            attnT = []
            for jt in range(CT):
                aT = smp.tile([P, 256], F16, tag="attnT")
                for it in range(CT):
                    tp = pst.tile([P, P], F16, tag="tp")
                    nc.tensor.transpose(
                        tp, attn[it][:, jt * P:(jt + 1) * P], ident)
                    nc.vector.tensor_copy(aT[:, it * P:(it + 1) * P], tp)
                attnT.append(aT)

        # ---- A'^T = wv' attn^T, pre-scaled by gamma ----
        # a [128, 512] = [A^T(c0) | A^T(c1)], arow [2,256] = [(attn bv)^T; 0]
        with nc.named_scope("attn_wv"):
            ap_ = pss.tile([P, 2 * 256], F32, tag="big")
            for mt in range(CT):
                for jt in range(CT):
                    nc.tensor.matmul(
                        ap_[:, mt * 256:(mt + 1) * 256],
                        wv_t[jt][:, mt * P:(mt + 1) * P], attnT[jt],
                        start=(jt == 0), stop=(jt == 1))
            arow = pss.tile([2, 256], F32, tag="big")
            for jt in range(CT):
                nc.tensor.matmul(arow, wv_t[jt][:, 256:258], attnT[jt],
                                 start=(jt == 0), stop=(jt == 1))
            at_s = []
            for mt in range(CT):
                t = gsb.tile([P, 256], F16, tag="ats")
                nc.vector.tensor_scalar_mul(
                    t, ap_[:, mt * 256:(mt + 1) * 256], gamma_col)
                at_s.append(t)
            abv_r = smp.tile([2, 256], F16, tag="abvr")
            nc.vector.tensor_scalar_mul(abv_r, arow, gamma_col[0:2, :])
            gabv = []
            for it in range(CT):
                tp = pst.tile([P, 2], F16, tag="tp")
                nc.tensor.transpose(
                    tp, abv_r[0:2, it * P:(it + 1) * P], ident[0:2, 0:2])
                gc = smp.tile([P, 1], F32, tag="gabv")
                nc.vector.tensor_copy(gc, tp[:, 0:1])
                gabv.append(gc)

        # ---- out = gamma*(A x) ; y = out + gamma*abv + x ----
        with nc.named_scope("out_mm"):
            for it in range(CT):
                for yg in range(N // YCOLS):
                    ysb = ysp.tile([P, YCOLS], F32, tag="ysb")
                    for sub in range(YCOLS // FD):
                        nch = yg * (YCOLS // FD) + sub
                        op = pso.tile([P, FD], F32, tag="out")
                        for ct in range(CT):
                            nc.tensor.matmul(
                                op, at_s[ct][:, it * P:(it + 1) * P],
                                xb[ct][:, nch * FD:(nch + 1) * FD],
                                start=(ct == 0), stop=(ct == CT - 1))
                        # y = (psum + gamma*abv) + x   (one DVE op)
                        nc.vector.scalar_tensor_tensor(
                            out=ysb[:, sub * FD:(sub + 1) * FD],
                            in0=op, scalar=gabv[it],
                            in1=xb[it][:, nch * FD:(nch + 1) * FD],
                            op0=mybir.AluOpType.add,
                            op1=mybir.AluOpType.add)
                    nc.sync.dma_start(
                        out=y_out[b, it * P:(it + 1) * P,
                                  yg * YCOLS:(yg + 1) * YCOLS],
                        in_=ysb)


_CACHE = {}
LAST_RESULTS = None


def _build():
    if "nc" in _CACHE:
        return _CACHE["nc"]
    nc = bacc.Bacc()
    x_in = nc.declare_dram_parameter("x", [PB, C, N], F16, isOutput=False)
    wpack = nc.declare_dram_parameter("wpack", [P, WCOLS], F16,
                                      isOutput=False)
    y_out = nc.declare_dram_parameter("y", [PB, C, N], F32, isOutput=True)
    with ExitStack() as ctx:
        tc = ctx.enter_context(tile.TileContext(nc))
        _emit_core_program(nc, tc, ctx, x_in, wpack, y_out)
    nc.compile()
    _CACHE["nc"] = nc
    return nc


def _pack_weights(wq, bq, wk, bk, wv, bv, gamma):
    wp = np.zeros((P, WCOLS), np.float16)
    wqT = np.ascontiguousarray(wq.T).astype(np.float16)
    wkT = np.ascontiguousarray(wk.T).astype(np.float16)
    wp[:, _WQ0:_WQ0 + 256] = wqT[0:P]
    wp[:, _WQ1:_WQ1 + 256] = wqT[P:C]
    wp[:, _WK0:_WK0 + 256] = wkT[0:P]
    wp[:, _WK1:_WK1 + 256] = wkT[P:C]
    wvp = np.concatenate([wv, bv[:, None]],
                         axis=1).astype(np.float16)  # [256, 257]
    wp[:, _WV0:_WV0 + 257] = wvp[0:P]
    wp[:, _WV1:_WV1 + 257] = wvp[P:C]
    wp[0, _BQ:_BQ + 256] = bq.astype(np.float16)
    wp[0, _BK:_BK + 256] = bk.astype(np.float16)
    wp[:, _GA] = np.float16(gamma)
    wp[:, _ONE] = np.float16(1.0)
    wp[0, _NC] = np.float16(float(N))
    return wp


def kernel(x, wq, bq, wk, bk, wv, bv, gamma):
    global LAST_RESULTS
    x = np.asarray(x, np.float32)
    x16 = np.ascontiguousarray(x.reshape(B, C, N).astype(np.float16))
    wp = _pack_weights(np.asarray(wq, np.float32), np.asarray(bq, np.float32),
                       np.asarray(wk, np.float32), np.asarray(bk, np.float32),
                       np.asarray(wv, np.float32), np.asarray(bv, np.float32),
                       np.asarray(gamma, np.float32).reshape(-1)[0])
    nc = _build()
    in_maps = []
    for k in range(NCORES):
        in_maps.append({
            "x": np.ascontiguousarray(x16[k * PB:(k + 1) * PB]),
            "wpack": wp,
        })
    trace = bool(int(os.environ.get("KERNEL_TRACE", "0")))
    res = run_bass_kernel_spmd(nc, in_maps, core_ids=list(range(NCORES)),
                               trace=trace)
    LAST_RESULTS = res
    y = np.concatenate([res.results[k]["y"][None] for k in range(NCORES)],
                       axis=0)
    return y.reshape(B, C, H, W).astype(np.float32)
